# revision 7
# baseline (speedup 1.0000x reference)
"""Trainium2 Bass kernel for nn_Conv2d_72430328481302.

Conv2d: input (16,128,56,56) f32, weight (128,128,3,3), bias (128),
stride 1, pad 1, dilation 1 -> output (16,128,56,56).

Strategy (Winograd F(2,3) along H):
  - Data-parallel over batch: 2 images per core across 8 cores, weight
    replicated.  Host pre-pads each image to a [Cin=128, 58x58]
    zero-framed bf16 plane so input DMA is contiguous row chunks.
  - The 3x3 conv is decomposed as Winograd F(2,3) along H (tap kh
    eliminated) x direct along W: for each pair of output rows
    (2t, 2t+1) the DVE computes four transformed rows from padded
    input rows r = P[2t..2t+3]:
        V0 = r0 - r2, V1 = r1 + r2, V2 = r2 - r1, V3 = r1 - r3
    and the PE accumulates, per j in 0..3, M_j = sum_kw U[j,kw] @
    V_j[:, kw:kw+56] (3 matmuls each, f32 PSUM).  U[j,kw] =
    sum_kh G[j,kh] w[:,:,kh,kw] is host-precomputed (G is the F(2,3)
    weight transform).  Output rows come from cheap elementwise ops:
        y[2t]   = M0 + M1 + M2 + b
        y[2t+1] = M1 - M2 - M3 + b
    This runs 12 matmul-columns per 2 output rows instead of direct
    conv's 18 - a 1.5x cut in PE time (the baseline's bottleneck; its
    PE ran saturated, gaps ~300ns total).
  - Per 8-pair slab (16 output rows, 448 psum cols): 12 accumulating
    bf16 matmuls into 4 PSUM banks (2 slabs in flight = 8 banks), then
    the transform spread across the idle engines so it hides under the
    PE: scalar ACT s0=M0+b, s1=-M3+b; vector stt t0=M1+s0, t1=M1+s1
    (+ the V ops for upcoming slabs); gpsimd Y0=t0+M2 -> even rows,
    Y1=t1-M2 -> odd rows (strided bf16 row writes).
  - Images split into pair-chunks 4,8,8,8 / 8,8,8,4 so the first real
    matmul group is small (starts sooner at DVFS ramp) and the last is
    small (short drain tail).  Warmup matmuls on raw scratch keep the
    PE busy from preamble-end to ramp the DVFS clock (as in the
    baseline); inputs ride the sync ring, weights lead on the scalar
    ring, outputs go out per-slab on the scalar ring (sync for the
    tail).
"""

import os
import sys

for _p in ("/opt/trn_rl_repo",):
    if os.path.isdir(_p) and _p not in sys.path:
        sys.path.insert(0, _p)

import ml_dtypes
import numpy as np

import concourse.bass as bass
import concourse.tile as tile
from concourse import bacc, mybir
from concourse.bass_utils import run_bass_kernel_spmd

N_CORES = 8
N_IMGS = 16
IPC = N_IMGS // N_CORES  # images per core
CIN = 128
COUT = 128
H = W = 56
WP = 58  # padded width (1 col each side)
HP = 58  # padded height (1 row each side)
FLATP = HP * WP  # 3364
# pad the per-partition plane so the V-transform's strided row views
# (last pair reads up to offset (2*24+3)*58 + 4*116 = 3422) stay in
# bounds; 3424 bf16 = 6848 B, a 32 B multiple
PAD_ALLOC = 3424
NPAIR = H // 2  # 28 output row-pairs per image
VROW = WP  # transformed row width
VPLANE = NPAIR * VROW  # 1624 cols per j-plane
OUT_ALLOC = H * W + W  # odd-row strided view needs 56 cols of slack
F32 = mybir.dt.float32
BF16 = mybir.dt.bfloat16

# pair-chunks (p0, npairs) per image: small first chunk (earlier PE
# start during DVFS ramp), small last chunk (short drain tail)
SLABS = [
    (0, 0, 4), (0, 4, 8), (0, 12, 8), (0, 20, 8),
    (1, 0, 8), (1, 8, 8), (1, 16, 8), (1, 24, 4),
]

_CACHE = {}


def _build_nc():
    nc = bacc.Bacc(
        "TRN2",
        target_bir_lowering=False,
        debug=False,
        num_devices=N_CORES,
    )
    x = nc.dram_tensor("x", [IPC, CIN, PAD_ALLOC], BF16, kind="ExternalInput")
    wt = nc.dram_tensor("wt", [CIN, 12, COUT], BF16, kind="ExternalInput")
    bvec = nc.dram_tensor("bvec", [COUT, 1], F32, kind="ExternalInput")
    # output travels as bf16 (halves store wire time; host casts back to
    # f32 - adds ~0.2% RMS, far under the 2e-2 gate)
    y = nc.dram_tensor("y", [IPC, COUT, H * W], BF16, kind="ExternalOutput")

    # raw (non-pool) scratch for PE warmup: outside the tile framework
    # there is no written-before-read requirement, so the first warmup
    # matmul issues right after the PE preamble with no memset gate;
    # the garbage values never leave PSUM (start=True resets each time)
    scrw = nc.alloc_sbuf_tensor("scrw", [CIN, 576], BF16)

    def row_view(ap_flat, off, nrows, pitch, width):
        # [c, nrows, width] strided-row view of a flat [c, N] AP
        return ap_flat[:, off : off + nrows * pitch].rearrange(
            "c (r k) -> c r k", k=pitch
        )[:, :, 0:width]

    with tile.TileContext(nc) as tc:
        with (
            tc.tile_pool(name="const", bufs=1) as cpool,
            tc.tile_pool(name="xin", bufs=1) as xpool,
            tc.tile_pool(name="vtr", bufs=1) as vpool,
            tc.tile_pool(name="yout", bufs=1) as ypool,
            tc.tile_pool(name="evac", bufs=10) as epool,
            tc.tile_pool(name="psum", bufs=8, space="PSUM") as pspool,
        ):
            # PE warmup: matmuls on raw scratch SBUF (no deps at all, so
            # they run right after the engine preamble and keep the PE
            # busy - ramping the DVFS clock - while the first
            # input/weight DMAs land)
            wps = pspool.tile([COUT, 448], F32, name="wps", tag="ps")
            for wi in range(7):
                nc.tensor.matmul(
                    wps[:],
                    scrw.ap()[:, 0:128],
                    scrw.ap()[:, 128:576],
                    start=True, stop=True,
                )
            for wi in range(4):
                nc.tensor.matmul(
                    wps[:, 0:128],
                    scrw.ap()[:, 0:128],
                    scrw.ap()[:, 128:256],
                    start=True, stop=True,
                )

            wt_sb = cpool.tile([CIN, 12, COUT], BF16, name="wt_sb", tag="wt_sb")
            bias_sb = cpool.tile([COUT, 1], F32, name="bias_sb", tag="bias_sb")
            P = {}
            V = {}
            out_sb = {}
            for i in range(IPC):
                P[i] = xpool.tile(
                    [CIN, PAD_ALLOC], BF16, name=f"P{i}", tag=f"P{i}"
                )
                V[i] = vpool.tile(
                    [CIN, 4, VPLANE], BF16, name=f"V{i}", tag=f"V{i}"
                )
                out_sb[i] = ypool.tile(
                    [COUT, OUT_ALLOC], BF16, name=f"out{i}", tag=f"out{i}"
                )

            # j=0 weight taps lead on the scalar ring: they gate the
            # first matmul group; input data must NOT ride the scalar
            # ring (it lands late behind sync traffic)
            nc.scalar.dma_start(wt_sb[:, 0:3, :], wt.ap()[:, 0:3, :])
            nc.scalar.dma_start(wt_sb[:, 3:6, :], wt.ap()[:, 3:6, :])
            nc.scalar.dma_start(wt_sb[:, 6:9, :], wt.ap()[:, 6:9, :])
            nc.scalar.dma_start(wt_sb[:, 9:12, :], wt.ap()[:, 9:12, :])
            nc.scalar.dma_start(bias_sb[:], bvec.ap()[:])

            # all input chunks issue up-front on the sync ring; finer
            # chunks bound the cost of a late piece when the DMA clock
            # is in its slow regime
            CHUNKS = [(0, 18), (18, 26), (26, 34), (34, 42), (42, 50), (50, HP)]
            for i in range(IPC):
                for r0, r1 in CHUNKS:
                    e0 = r0 * WP
                    e1 = r1 * WP if r1 < HP else PAD_ALLOC
                    nc.sync.dma_start(P[i][:, e0:e1], x.ap()[i, :, e0:e1])

            def v_ops(k):
                # DVE input transform for slab k: 4 strided row combines
                i, p0, npr = SLABS[k]
                dsts = [
                    row_view(V[i][:, j, :], p0 * VROW, npr, VROW, VROW)
                    for j in range(4)
                ]
                r = [
                    row_view(P[i], (2 * p0 + d) * WP, npr, 2 * WP, WP)
                    for d in range(4)
                ]
                nc.vector.tensor_sub(dsts[0], r[0], r[2])
                nc.vector.tensor_add(dsts[1], r[1], r[2])
                nc.vector.tensor_sub(dsts[2], r[2], r[1])
                nc.vector.tensor_sub(dsts[3], r[1], r[3])

            # prime the V pipeline one slab ahead of the matmuls
            v_ops(0)
            v_ops(1)

            pend_dma = []  # (engine, dram_ap, sbuf_ap) awaiting issue
            for k, (i, p0, npr) in enumerate(SLABS):
                ncols = npr * W
                ps = [
                    pspool.tile([COUT, ncols], F32, name=f"ps{k}_{j}", tag="ps")
                    for j in range(4)
                ]
                for j in range(4):
                    vv = row_view(V[i][:, j, :], p0 * VROW, npr, VROW, VROW)
                    for kw in range(3):
                        nc.tensor.matmul(
                            ps[j][:],
                            wt_sb[:, 3 * j + kw, :],
                            vv[:, :, kw : kw + W],
                            start=(kw == 0),
                            stop=(kw == 2),
                        )

                # output transform: y[2t] = M0+M1+M2+b on even rows,
                # y[2t+1] = M1-M2-M3+b on odd rows.  Only scalar and
                # vector can read PSUM (gpsimd has no PSUM port):
                # scalar evacuates M0/M3 (bias folded) and M2, vector
                # reads M1 twice, gpsimd does the SBUF-only combines.
                s0 = epool.tile([COUT, ncols], BF16, name=f"s0_{k}", tag="ev")
                s1 = epool.tile([COUT, ncols], BF16, name=f"s1_{k}", tag="ev")
                m2 = epool.tile([COUT, ncols], BF16, name=f"m2_{k}", tag="ev")
                t0 = epool.tile([COUT, ncols], BF16, name=f"t0_{k}", tag="ev")
                t1 = epool.tile([COUT, ncols], BF16, name=f"t1_{k}", tag="ev")
                nc.scalar.activation(
                    s0[:], ps[0][:],
                    mybir.ActivationFunctionType.Identity,
                    bias=bias_sb[:, :],
                )
                nc.scalar.activation(
                    s1[:], ps[3][:],
                    mybir.ActivationFunctionType.Identity,
                    bias=bias_sb[:, :], scale=-1.0,
                )
                nc.scalar.activation(
                    m2[:], ps[2][:],
                    mybir.ActivationFunctionType.Identity,
                )
                nc.vector.tensor_add(t0[:], ps[1][:], s0[:])
                nc.vector.tensor_add(t1[:], ps[1][:], s1[:])
                # queue the V ops for slab k+2 behind this slab's vector
                # work so they don't delay t0/t1 but stay a slab ahead
                # of the PE
                if k + 2 < len(SLABS):
                    v_ops(k + 2)
                c0 = 2 * p0 * W
                y0 = row_view(out_sb[i], c0, npr, 2 * W, W)
                y1 = row_view(out_sb[i], c0 + W, npr, 2 * W, W)
                m2v = m2[:].rearrange("c (r k) -> c r k", k=W)
                t0v = t0[:].rearrange("c (r k) -> c r k", k=W)
                t1v = t1[:].rearrange("c (r k) -> c r k", k=W)
                nc.gpsimd.tensor_add(y0, t0v, m2v)
                nc.gpsimd.tensor_sub(y1, t1v, m2v)

                # store the previous slab now (its Y deps are long done,
                # so the ring isn't stalled on an evac wait)
                pend_dma.append((y.ap()[i, :, c0 : c0 + ncols * 2],
                                 out_sb[i][:, c0 : c0 + ncols * 2]))
                if len(pend_dma) > 1:
                    dst, src = pend_dma.pop(0)
                    nc.scalar.dma_start(dst, src)

            # final slab's store: halves crossed over the sync (idle
            # after inputs) and scalar rings to halve the drain tail
            dst, src = pend_dma.pop(0)
            half = 2 * SLABS[-1][2] * W // 2
            nc.sync.dma_start(dst[:, 0:half], src[:, 0:half],
                              single_packet=True)
            nc.scalar.dma_start(dst[:, half:], src[:, half:],
                                single_packet=True)

    nc.compile()
    return nc


def _get_nc():
    if "nc" not in _CACHE:
        _CACHE["nc"] = _build_nc()
    return _CACHE["nc"]


def _make_in_maps(input, weight, bias):
    input = np.asarray(input)
    weight = np.asarray(weight)
    bias = np.asarray(bias)
    # pad every image into the [IPC, CIN, 58*58 (+pad)] zero-framed plane
    padded = np.zeros((N_IMGS, CIN, PAD_ALLOC), dtype=ml_dtypes.bfloat16)
    pv = padded[:, :, :FLATP].reshape(N_IMGS, CIN, HP, WP)
    pv[:, :, 1 : H + 1, 1 : W + 1] = input
    # weight (Cout,Cin,3,3) -> Winograd F(2,3) transform along kh, then
    # lhsT layout (Cin, j*3+kw, Cout)
    G = np.array(
        [[1, 0, 0], [0.5, 0.5, 0.5], [0.5, -0.5, 0.5], [0, 0, 1]],
        dtype=np.float32,
    )
    U = np.einsum("jh,oihw->ijwo", G,
                  weight.astype(np.float32))  # [Cin, 4, 3, Cout]
    wt_host = np.ascontiguousarray(
        U.reshape(CIN, 12, COUT)
    ).astype(ml_dtypes.bfloat16)
    b_host = np.ascontiguousarray(bias.reshape(COUT, 1), dtype=np.float32)
    return [
        {
            "x": padded[c * IPC : (c + 1) * IPC],
            "wt": wt_host,
            "bvec": b_host,
        }
        for c in range(N_CORES)
    ]


def run(input, weight, bias, trace=False, tmpdir=None):
    """Run the SPMD kernel; returns (output, BassKernelResults)."""
    nc = _get_nc()
    in_maps = _make_in_maps(input, weight, bias)
    res = run_bass_kernel_spmd(
        nc, in_maps, list(range(N_CORES)), trace=trace, tmpdir=tmpdir
    )
    out = np.concatenate(
        [np.asarray(res.results[c]["y"]) for c in range(N_CORES)], axis=0
    ).astype(np.float32)
    return out.reshape(N_IMGS, COUT, H, W).astype(np.float32), res


def kernel(input, weight, bias):
    out, _ = run(input, weight, bias, trace=False)
    return out
